# revision 14
# baseline (speedup 1.0000x reference)
"""Trainium2 Bass kernel for nn_Attention_49134425866421.

Dense transformer attention block:
  qkv = x @ W_qkv + b_qkv  -> partial RoPE on q,k -> softmax attention -> out proj.

Shapes (hardcoded): B=4, N=2048, C=768, H=12, D=64, fp32.

Sharding: 8 cores = (batch b in 0..3) x (head-group g in 0..1, 6 heads each).
Each core computes q/k/v projections for its 6 heads, attention, and a partial
output projection (row-parallel over head dims). Host sums the two partials
per batch and adds b_proj.

On-chip layouts (per core):
  xT    [128,6,2048]  x[b]^T, contraction dim c on partitions (c = ko*128+p)
  qT,kT [128,3,2048]  per-head-pair: partition p = 64*(h%2)+d, free (hp, t)
  V     [128,16,6,65] natural: partition = t%128, free (t//128, local head, d)
                      column 64 holds ones -> AV matmul also produces rowsums
  attnT [128,3,2048]  bf16, same layout as qT -> feeds row-parallel proj

RoPE trick: rotate_half is a cross-partition half-swap; done via SBUF->SBUF
DMA of (q * m2s) where m2s = pre-swapped signed sin table, so
q_rope = q*cos + swap(q*m2s). Special (non-rotated) tokens handled by padding
cos=1,sin=0 rows host-side. Softmax without max-subtraction (scores are
N(0,~1); exp never overflows); scale 1/8 folded into the ACT exp call;
rowsum via the ones-column of V'.
"""

import os
import sys

import numpy as np

try:
    import concourse.bass as bass  # noqa: F401
except ImportError:
    sys.path.insert(0, "/opt/trn_rl_repo")

import ml_dtypes

B, N, C, H, D = 4, 2048, 768, 12, 64
HPC = 6          # heads per core
NPAIR = 3        # head pairs per core
P = 128
NT = N // P      # 16 token tiles
TC = 512         # token chunk for matmul free dim
NTC = N // TC    # 4

_NC_CACHE = {}
LAST_RESULTS = None  # BassKernelResults stash for test.py


def _build_nc():
    from contextlib import ExitStack

    import concourse.bass as bass
    import concourse.bacc as bacc
    import concourse.mybir as mybir
    import concourse.tile as tile

    f32 = mybir.dt.float32
    f32r = mybir.dt.float32r
    bf16 = mybir.dt.bfloat16
    EXP = mybir.ActivationFunctionType.Exp

    nc = bacc.Bacc(None, target_bir_lowering=False)

    xT_d = nc.dram_tensor("xT", [C, N], f32r, kind="ExternalInput")
    wqk_d = nc.dram_tensor("w_qk", [P, 6, 768], f32r, kind="ExternalInput")
    wv_d = nc.dram_tensor("w_v", [P, 6, 384], f32r, kind="ExternalInput")
    wp_d = nc.dram_tensor("w_p", [P, 3, 768], bf16, kind="ExternalInput")
    bqk_d = nc.dram_tensor("b_qk", [1, 768], f32r, kind="ExternalInput")
    bv_d = nc.dram_tensor("b_v", [1, 384], f32r, kind="ExternalInput")
    ones_d = nc.dram_tensor("ones", [1, TC], f32r, kind="ExternalInput")
    cos_d = nc.dram_tensor("cos_tab", [P, N], f32, kind="ExternalInput")
    m2s_d = nc.dram_tensor("m2s_tab", [P, N], f32, kind="ExternalInput")
    y_d = nc.dram_tensor("y", [N, C], f32, kind="ExternalOutput")

    with tile.TileContext(nc) as tc, ExitStack() as ctx:
        singles = ctx.enter_context(tc.tile_pool(name="singles", bufs=1))
        mm_ps = ctx.enter_context(tc.tile_pool(name="mm_ps", bufs=2, space="PSUM"))
        att_ps = ctx.enter_context(tc.tile_pool(name="att_ps", bufs=2, space="PSUM"))
        acc_ps = ctx.enter_context(tc.tile_pool(name="acc_ps", bufs=1, space="PSUM"))
        rope_tmp = ctx.enter_context(tc.tile_pool(name="rope_tmp", bufs=2))
        pt_pool = ctx.enter_context(tc.tile_pool(name="pt", bufs=3))
        rb_pool = ctx.enter_context(tc.tile_pool(name="rb", bufs=2))
        y_pool = ctx.enter_context(tc.tile_pool(name="yout", bufs=2))

        # ---- static SBUF tensors ----
        xT = singles.tile([P, 6, N], f32r)
        wqk = singles.tile([P, 6, 768], f32r)
        wv = singles.tile([P, 6, 384], f32r)
        wp = singles.tile([P, 3, 768], bf16)
        bqk = singles.tile([1, 768], f32r)
        bv = singles.tile([1, 384], f32r)
        cosT = singles.tile([P, N], f32)
        m2sT = singles.tile([P, N], f32)
        ones = singles.tile([1, TC], f32r)
        qT = singles.tile([P, NPAIR, N], f32r)
        kT = singles.tile([P, NPAIR, N], f32r)
        Vt = singles.tile([P, NT, HPC, D + 1], bf16)
        attnT = singles.tile([P, NPAIR, N], bf16)

        xT_r = xT_d.rearrange("(ko p) t -> p ko t", p=P)
        for ko in range(6):
            nc.sync.dma_start(xT[:, ko, :], xT_r[:, ko, :])
        nc.scalar.dma_start(wqk[:], wqk_d[:])
        nc.sync.dma_start(wv[:], wv_d[:])
        nc.scalar.dma_start(bqk[:], bqk_d[:])
        nc.sync.dma_start(bv[:], bv_d[:])
        nc.sync.dma_start(ones[:], ones_d[:])
        nc.scalar.dma_start(cosT[:], cos_d[:])
        nc.scalar.dma_start(m2sT[:], m2s_d[:])
        nc.sync.dma_start(wp[:], wp_d[:])
        nc.gpsimd.memset(Vt[:], 1.0)

        def emit_qk(hp):
            for mt in (hp, 3 + hp):  # q pair then k pair
                dst = qT if mt < 3 else kT
                for tcu in range(NTC):
                    tsl = slice(tcu * TC, (tcu + 1) * TC)
                    ps = mm_ps.tile([P, TC], f32, tag="mm")
                    for ko in range(6):
                        nc.tensor.matmul(
                            ps,
                            lhsT=wqk[:, ko, mt * P : (mt + 1) * P],
                            rhs=xT[:, ko, tsl],
                            start=(ko == 0),
                            stop=False,
                        )
                    nc.tensor.matmul(
                        ps,
                        lhsT=bqk[:, mt * P : (mt + 1) * P],
                        rhs=ones[:],
                        start=False,
                        stop=True,
                    )
                    # rope: dst = ps*cos + swap(ps*m2s)
                    qs = rope_tmp.tile([P, TC], f32, tag="qs")
                    qsw = rope_tmp.tile([P, TC], f32, tag="qsw")
                    nc.vector.tensor_mul(out=qs[:], in0=ps[:], in1=m2sT[:, tsl])
                    nc.vector.tensor_mul(
                        out=dst[:, hp, tsl], in0=ps[:], in1=cosT[:, tsl]
                    )
                    for blk in range(4):
                        sp = [1, 0, 3, 2][blk] * 32
                        nc.sync.dma_start(
                            out=qsw[blk * 32 : blk * 32 + 32, :],
                            in_=qs[sp : sp + 32, :],
                        )
                    nc.vector.tensor_add(
                        out=dst[:, hp, tsl], in0=dst[:, hp, tsl], in1=qsw[:]
                    )

        emit_qk(0)

        # ---- V projection (natural layout), all 6 heads; emitted per
        # token-tile, interleaved into the first attention pass ----
        def emit_v(tt):
            ps = mm_ps.tile([P, TC], f32, tag="mm")
            vps = ps[:, :384]
            for ko in range(6):
                nc.tensor.matmul(
                    vps,
                    lhsT=xT[:, ko, tt * P : (tt + 1) * P],
                    rhs=wv[:, ko, :],
                    start=(ko == 0),
                    stop=False,
                )
            nc.tensor.matmul(
                vps, lhsT=ones[:, :P], rhs=bv[:], start=False, stop=True
            )
            nc.vector.tensor_copy(
                out=Vt[:, tt, :, :D],
                in_=vps.rearrange("p (h d) -> p h d", h=HPC),
            )

        # ---- per head-pair: q/k projection + RoPE, then attention ----
        for hp in range(NPAIR):
            if hp > 0:
                emit_qk(hp)

            # attention for the two heads of this pair
            for ic in range(NTC):
                isl = slice(ic * TC, (ic + 1) * TC)
                accA = acc_ps.tile([D + 1, TC], f32, tag="accA")
                accB = acc_ps.tile([D + 1, TC], f32, tag="accB")
                for jt in range(NT):
                    if hp == 0 and ic == 0:
                        emit_v(jt)
                    st = att_ps.tile([P, 2 * TC], f32, tag="st")
                    nc.tensor.matmul(
                        st[:, :TC],
                        lhsT=kT[:D, hp, jt * P : (jt + 1) * P],
                        rhs=qT[:D, hp, isl],
                        start=True,
                        stop=True,
                        tile_position=(0, 0),
                    )
                    nc.tensor.matmul(
                        st[:, TC:],
                        lhsT=kT[D:, hp, jt * P : (jt + 1) * P],
                        rhs=qT[D:, hp, isl],
                        start=True,
                        stop=True,
                        tile_position=(64, 0),
                    )
                    pt = pt_pool.tile([P, 2 * TC], bf16, tag="pt")
                    if os.environ.get("ABLATE") == "exp":
                        nc.scalar.activation(pt[:, :8], st[:, :8], EXP, scale=0.125)
                        nc.scalar.activation(pt[:, 8:], st[:, 8:], EXP, scale=0.125) if False else None
                    else:
                        nc.scalar.activation(pt[:], st[:], EXP, scale=0.125)
                    nc.tensor.matmul(
                        accA,
                        lhsT=Vt[:, jt, 2 * hp, :],
                        rhs=pt[:, :TC],
                        start=(jt == 0),
                        stop=(jt == NT - 1),
                    )
                    nc.tensor.matmul(
                        accB,
                        lhsT=Vt[:, jt, 2 * hp + 1, :],
                        rhs=pt[:, TC:],
                        start=(jt == 0),
                        stop=(jt == NT - 1),
                    )
                # evacuate PSUM accs to SBUF immediately so the banks free up
                # for the next i-chunk; rescale then runs off the PE critical
                # path entirely.
                accs = []
                for half, acc in ((0, accA), (1, accB)):
                    asb = rb_pool.tile([D + 1, TC], f32, tag="asb")
                    nc.vector.tensor_copy(out=asb[:], in_=acc[:])
                    accs.append(asb)
                for half, asb in ((0, accs[0]), (1, accs[1])):
                    rec = rb_pool.tile([1, TC], f32, tag="rec")
                    rbc = rb_pool.tile([D, TC], f32, tag="rbc")
                    nc.vector.reciprocal(out=rec[:], in_=asb[D : D + 1, :])
                    nc.gpsimd.partition_broadcast(rbc[:], rec[:], channels=D)
                    nc.vector.tensor_mul(
                        out=attnT[half * D : (half + 1) * D, hp, isl],
                        in0=asb[:D, :],
                        in1=rbc[:],
                    )

        # ---- output projection (row-parallel partial) ----
        for tt in range(NT):
            for ch in range(2):
                ps = mm_ps.tile([P, TC], f32, tag="mm")
                yps = ps[:, :384]
                for ko in range(3):
                    nc.tensor.matmul(
                        yps,
                        lhsT=attnT[:, ko, tt * P : (tt + 1) * P],
                        rhs=wp[:, ko, ch * 384 : (ch + 1) * 384],
                        start=(ko == 0),
                        stop=(ko == 2),
                    )
                yt = y_pool.tile([P, 384], f32, tag="yt")
                nc.vector.tensor_copy(out=yt[:], in_=yps)
                nc.sync.dma_start(
                    out=y_d[tt * P : (tt + 1) * P, ch * 384 : (ch + 1) * 384],
                    in_=yt[:],
                )

    nc.finalize()
    return nc


def _host_inputs(x, rope_cos, rope_sin, W_qkv, b_qkv, W_proj, b_proj, num_special):
    ns = int(num_special)
    cos_pad = np.ones((N, D), np.float32)
    sin_pad = np.zeros((N, D), np.float32)
    cos_pad[ns:] = rope_cos
    sin_pad[ns:] = rope_sin
    # m2s[t, d] = +sin[t, d+32] (d<32) else -sin[t, d-32]
    m2s = np.empty_like(sin_pad)
    m2s[:, : D // 2] = sin_pad[:, D // 2 :]
    m2s[:, D // 2 :] = -sin_pad[:, : D // 2]
    cos_tab = np.tile(np.ascontiguousarray(cos_pad.T), (2, 1))
    m2s_tab = np.tile(np.ascontiguousarray(m2s.T), (2, 1))

    in_maps = []
    for core in range(8):
        b, g = core // 2, core % 2
        hs = list(range(HPC * g, HPC * g + HPC))
        cols_qk = []
        for mt in range(6):
            s, hp = (0, mt) if mt < 3 else (1, mt - 3)
            for half in range(2):
                h = hs[2 * hp + half]
                cols_qk.extend(s * 768 + h * 64 + d for d in range(D))
        cols_qk = np.array(cols_qk)
        cols_v = np.array([2 * 768 + hs[i // 64] * 64 + (i % 64) for i in range(384)])
        rows_p = np.array(
            [hs[2 * ko + half] * 64 + d
             for ko in range(3) for half in range(2) for d in range(D)]
        )
        in_maps.append({
            "xT": np.ascontiguousarray(x[b].T),
            "w_qk": np.ascontiguousarray(
                W_qkv[:, cols_qk].reshape(6, P, 768).transpose(1, 0, 2)),
            "w_v": np.ascontiguousarray(
                W_qkv[:, cols_v].reshape(6, P, 384).transpose(1, 0, 2)),
            "w_p": np.ascontiguousarray(
                W_proj[rows_p].reshape(3, P, 768).transpose(1, 0, 2)
            ).astype(ml_dtypes.bfloat16),
            "b_qk": np.ascontiguousarray(b_qkv[cols_qk].reshape(1, 768)),
            "b_v": np.ascontiguousarray(b_qkv[cols_v].reshape(1, 384)),
            "ones": np.ones((1, TC), np.float32),
            "cos_tab": cos_tab,
            "m2s_tab": m2s_tab,
        })
    return in_maps


def kernel(x, rope_cos, rope_sin, W_qkv, b_qkv, W_proj, b_proj, num_special):
    global LAST_RESULTS
    from concourse.bass_utils import run_bass_kernel_spmd

    x = np.asarray(x, np.float32)
    if "nc" not in _NC_CACHE:
        _NC_CACHE["nc"] = _build_nc()
    nc = _NC_CACHE["nc"]

    in_maps = _host_inputs(
        x, np.asarray(rope_cos, np.float32), np.asarray(rope_sin, np.float32),
        np.asarray(W_qkv, np.float32), np.asarray(b_qkv, np.float32),
        np.asarray(W_proj, np.float32), np.asarray(b_proj, np.float32), num_special,
    )
    trace = bool(int(os.environ.get("KERNEL_TRACE", "0")))
    res = run_bass_kernel_spmd(nc, in_maps, core_ids=list(range(8)), trace=trace)
    LAST_RESULTS = res

    bp = np.asarray(b_proj, np.float32)
    out = np.empty((B, N, C), np.float32)
    for b in range(B):
        out[b] = res.results[2 * b]["y"] + res.results[2 * b + 1]["y"] + bp
    return out


# revision 15
# speedup vs baseline: 1.0177x; 1.0177x over previous
"""Trainium2 Bass kernel for nn_Attention_49134425866421.

Dense transformer attention block:
  qkv = x @ W_qkv + b_qkv  -> partial RoPE on q,k -> softmax attention -> out proj.

Shapes (hardcoded): B=4, N=2048, C=768, H=12, D=64, fp32.

Sharding: 8 cores = (batch b in 0..3) x (head-group g in 0..1, 6 heads each).
Each core computes q/k/v projections for its 6 heads, attention, and a partial
output projection (row-parallel over head dims). Host sums the two partials
per batch and adds b_proj.

On-chip layouts (per core):
  xT    [128,6,2048]  x[b]^T, contraction dim c on partitions (c = ko*128+p)
  qT,kT [128,3,2048]  per-head-pair: partition p = 64*(h%2)+d, free (hp, t)
  V     [128,16,6,65] natural: partition = t%128, free (t//128, local head, d)
                      column 64 holds ones -> AV matmul also produces rowsums
  attnT [128,3,2048]  bf16, same layout as qT -> feeds row-parallel proj

RoPE trick: rotate_half is a cross-partition half-swap; done via SBUF->SBUF
DMA of (q * m2s) where m2s = pre-swapped signed sin table, so
q_rope = q*cos + swap(q*m2s). Special (non-rotated) tokens handled by padding
cos=1,sin=0 rows host-side. Softmax without max-subtraction (scores are
N(0,~1); exp never overflows); scale 1/8 folded into the ACT exp call;
rowsum via the ones-column of V'.
"""

import os
import sys

import numpy as np

try:
    import concourse.bass as bass  # noqa: F401
except ImportError:
    sys.path.insert(0, "/opt/trn_rl_repo")

import ml_dtypes

B, N, C, H, D = 4, 2048, 768, 12, 64
HPC = 6          # heads per core
NPAIR = 3        # head pairs per core
P = 128
NT = N // P      # 16 token tiles
TC = 512         # token chunk for matmul free dim
NTC = N // TC    # 4

_NC_CACHE = {}
LAST_RESULTS = None  # BassKernelResults stash for test.py


def _build_nc():
    from contextlib import ExitStack

    import concourse.bass as bass
    import concourse.bacc as bacc
    import concourse.mybir as mybir
    import concourse.tile as tile

    f32 = mybir.dt.float32
    f32r = mybir.dt.float32r
    bf16 = mybir.dt.bfloat16
    EXP = mybir.ActivationFunctionType.Exp

    nc = bacc.Bacc(None, target_bir_lowering=False)

    xT_d = nc.dram_tensor("xT", [C, N], f32r, kind="ExternalInput")
    wqk_d = nc.dram_tensor("w_qk", [P, 6, 768], f32r, kind="ExternalInput")
    wv_d = nc.dram_tensor("w_v", [P, 6, 384], f32r, kind="ExternalInput")
    wp_d = nc.dram_tensor("w_p", [P, 3, 768], bf16, kind="ExternalInput")
    bqk_d = nc.dram_tensor("b_qk", [1, 768], f32r, kind="ExternalInput")
    bv_d = nc.dram_tensor("b_v", [1, 384], f32r, kind="ExternalInput")
    ones_d = nc.dram_tensor("ones", [1, TC], f32r, kind="ExternalInput")
    bqkt_d = nc.dram_tensor("b_qk_t", [P, 6], f32, kind="ExternalInput")
    cos_d = nc.dram_tensor("cos_tab", [P, N], f32, kind="ExternalInput")
    m2s_d = nc.dram_tensor("m2s_tab", [P, N], f32, kind="ExternalInput")
    y_d = nc.dram_tensor("y", [N, C], f32, kind="ExternalOutput")

    with tile.TileContext(nc) as tc, ExitStack() as ctx:
        singles = ctx.enter_context(tc.tile_pool(name="singles", bufs=1))
        mm_ps = ctx.enter_context(tc.tile_pool(name="mm_ps", bufs=2, space="PSUM"))
        att_ps = ctx.enter_context(tc.tile_pool(name="att_ps", bufs=2, space="PSUM"))
        acc_ps = ctx.enter_context(tc.tile_pool(name="acc_ps", bufs=1, space="PSUM"))
        rope_tmp = ctx.enter_context(tc.tile_pool(name="rope_tmp", bufs=2))
        pt_pool = ctx.enter_context(tc.tile_pool(name="pt", bufs=3))
        rb_pool = ctx.enter_context(tc.tile_pool(name="rb", bufs=2))
        y_pool = ctx.enter_context(tc.tile_pool(name="yout", bufs=2))

        # ---- static SBUF tensors ----
        xT = singles.tile([P, 6, N], f32r)
        wqk = singles.tile([P, 6, 768], f32r)
        wv = singles.tile([P, 6, 384], f32r)
        wp = singles.tile([P, 3, 768], bf16)
        bqk = singles.tile([1, 768], f32r)
        bv = singles.tile([1, 384], f32r)
        cosT = singles.tile([P, N], f32)
        m2sT = singles.tile([P, N], f32)
        ones = singles.tile([1, TC], f32r)
        bqkt = singles.tile([P, 6], f32)
        qT = singles.tile([P, NPAIR, N], f32r)
        kT = singles.tile([P, NPAIR, N], f32r)
        Vt = singles.tile([P, NT, HPC, D + 1], bf16)
        attnT = singles.tile([P, NPAIR, N], bf16)

        xT_r = xT_d.rearrange("(ko p) t -> p ko t", p=P)
        for ko in range(6):
            nc.sync.dma_start(xT[:, ko, :], xT_r[:, ko, :])
        nc.scalar.dma_start(wqk[:], wqk_d[:])
        nc.sync.dma_start(wv[:], wv_d[:])
        nc.scalar.dma_start(bqk[:], bqk_d[:])
        nc.sync.dma_start(bv[:], bv_d[:])
        nc.sync.dma_start(ones[:], ones_d[:])
        nc.sync.dma_start(bqkt[:], bqkt_d[:])
        nc.scalar.dma_start(cosT[:], cos_d[:])
        nc.scalar.dma_start(m2sT[:], m2s_d[:])
        nc.gpsimd.memset(Vt[:], 1.0)

        def emit_qk(hp):
            for tcu in range(NTC):
                tsl = slice(tcu * TC, (tcu + 1) * TC)
                for mt in (3 + hp, hp):  # k pair first, then q pair
                    dst = qT if mt < 3 else kT
                    ps = mm_ps.tile([P, TC], f32, tag="mm")
                    for ko in range(6):
                        nc.tensor.matmul(
                            ps,
                            lhsT=wqk[:, ko, mt * P : (mt + 1) * P],
                            rhs=xT[:, ko, tsl],
                            start=(ko == 0),
                            stop=(ko == 5),
                        )
                    # bias add on DVE, then rope: dst = pb*cos + swap(pb*m2s)
                    pb = rope_tmp.tile([P, TC], f32, tag="pb")
                    qs = rope_tmp.tile([P, TC], f32, tag="qs")
                    qsw = rope_tmp.tile([P, TC], f32, tag="qsw")
                    nc.vector.tensor_scalar_add(
                        out=pb[:], in0=ps[:], scalar1=bqkt[:, mt : mt + 1]
                    )
                    nc.vector.tensor_mul(out=qs[:], in0=pb[:], in1=m2sT[:, tsl])
                    nc.vector.tensor_mul(
                        out=dst[:, hp, tsl], in0=pb[:], in1=cosT[:, tsl]
                    )
                    for blk in range(4):
                        sp = [1, 0, 3, 2][blk] * 32
                        nc.sync.dma_start(
                            out=qsw[blk * 32 : blk * 32 + 32, :],
                            in_=qs[sp : sp + 32, :],
                        )
                    nc.vector.tensor_add(
                        out=dst[:, hp, tsl], in0=dst[:, hp, tsl], in1=qsw[:]
                    )

        emit_qk(0)
        nc.sync.dma_start(wp[:], wp_d[:])

        # ---- V projection (natural layout), all 6 heads; emitted per
        # token-tile, interleaved into the first attention pass ----
        def emit_v(tt):
            ps = mm_ps.tile([P, TC], f32, tag="mm")
            vps = ps[:, :384]
            for ko in range(6):
                nc.tensor.matmul(
                    vps,
                    lhsT=xT[:, ko, tt * P : (tt + 1) * P],
                    rhs=wv[:, ko, :],
                    start=(ko == 0),
                    stop=False,
                )
            nc.tensor.matmul(
                vps, lhsT=ones[:, :P], rhs=bv[:], start=False, stop=True
            )
            nc.vector.tensor_copy(
                out=Vt[:, tt, :, :D],
                in_=vps.rearrange("p (h d) -> p h d", h=HPC),
            )

        # ---- per head-pair: q/k projection + RoPE, then attention ----
        for hp in range(NPAIR):
            if hp > 0:
                emit_qk(hp)

            # attention for the two heads of this pair
            for ic in range(NTC):
                isl = slice(ic * TC, (ic + 1) * TC)
                accA = acc_ps.tile([D + 1, TC], f32, tag="accA")
                accB = acc_ps.tile([D + 1, TC], f32, tag="accB")
                for jt in range(NT):
                    if hp == 0 and ic == 0:
                        emit_v(jt)
                    st = att_ps.tile([P, 2 * TC], f32, tag="st")
                    nc.tensor.matmul(
                        st[:, :TC],
                        lhsT=kT[:D, hp, jt * P : (jt + 1) * P],
                        rhs=qT[:D, hp, isl],
                        start=True,
                        stop=True,
                        tile_position=(0, 0),
                    )
                    nc.tensor.matmul(
                        st[:, TC:],
                        lhsT=kT[D:, hp, jt * P : (jt + 1) * P],
                        rhs=qT[D:, hp, isl],
                        start=True,
                        stop=True,
                        tile_position=(64, 0),
                    )
                    pt = pt_pool.tile([P, 2 * TC], bf16, tag="pt")
                    if os.environ.get("ABLATE") == "exp":
                        nc.scalar.activation(pt[:, :8], st[:, :8], EXP, scale=0.125)
                        nc.scalar.activation(pt[:, 8:], st[:, 8:], EXP, scale=0.125) if False else None
                    else:
                        nc.scalar.activation(pt[:], st[:], EXP, scale=0.125)
                    nc.tensor.matmul(
                        accA,
                        lhsT=Vt[:, jt, 2 * hp, :],
                        rhs=pt[:, :TC],
                        start=(jt == 0),
                        stop=(jt == NT - 1),
                    )
                    nc.tensor.matmul(
                        accB,
                        lhsT=Vt[:, jt, 2 * hp + 1, :],
                        rhs=pt[:, TC:],
                        start=(jt == 0),
                        stop=(jt == NT - 1),
                    )
                # evacuate PSUM accs to SBUF immediately so the banks free up
                # for the next i-chunk; rescale then runs off the PE critical
                # path entirely.
                accs = []
                for half, acc in ((0, accA), (1, accB)):
                    asb = rb_pool.tile([D + 1, TC], f32, tag="asb")
                    nc.vector.tensor_copy(out=asb[:], in_=acc[:])
                    accs.append(asb)
                for half, asb in ((0, accs[0]), (1, accs[1])):
                    rec = rb_pool.tile([1, TC], f32, tag="rec")
                    rbc = rb_pool.tile([D, TC], f32, tag="rbc")
                    nc.vector.reciprocal(out=rec[:], in_=asb[D : D + 1, :])
                    nc.gpsimd.partition_broadcast(rbc[:], rec[:], channels=D)
                    nc.vector.tensor_mul(
                        out=attnT[half * D : (half + 1) * D, hp, isl],
                        in0=asb[:D, :],
                        in1=rbc[:],
                    )

        # ---- output projection (row-parallel partial) ----
        for tt in range(NT):
            for ch in range(2):
                ps = mm_ps.tile([P, TC], f32, tag="mm")
                yps = ps[:, :384]
                for ko in range(3):
                    nc.tensor.matmul(
                        yps,
                        lhsT=attnT[:, ko, tt * P : (tt + 1) * P],
                        rhs=wp[:, ko, ch * 384 : (ch + 1) * 384],
                        start=(ko == 0),
                        stop=(ko == 2),
                    )
                yt = y_pool.tile([P, 384], f32, tag="yt")
                nc.vector.tensor_copy(out=yt[:], in_=yps)
                nc.sync.dma_start(
                    out=y_d[tt * P : (tt + 1) * P, ch * 384 : (ch + 1) * 384],
                    in_=yt[:],
                )

    nc.finalize()
    return nc


def _host_inputs(x, rope_cos, rope_sin, W_qkv, b_qkv, W_proj, b_proj, num_special):
    ns = int(num_special)
    cos_pad = np.ones((N, D), np.float32)
    sin_pad = np.zeros((N, D), np.float32)
    cos_pad[ns:] = rope_cos
    sin_pad[ns:] = rope_sin
    # m2s[t, d] = +sin[t, d+32] (d<32) else -sin[t, d-32]
    m2s = np.empty_like(sin_pad)
    m2s[:, : D // 2] = sin_pad[:, D // 2 :]
    m2s[:, D // 2 :] = -sin_pad[:, : D // 2]
    cos_tab = np.tile(np.ascontiguousarray(cos_pad.T), (2, 1))
    m2s_tab = np.tile(np.ascontiguousarray(m2s.T), (2, 1))

    in_maps = []
    for core in range(8):
        b, g = core // 2, core % 2
        hs = list(range(HPC * g, HPC * g + HPC))
        cols_qk = []
        for mt in range(6):
            s, hp = (0, mt) if mt < 3 else (1, mt - 3)
            for half in range(2):
                h = hs[2 * hp + half]
                cols_qk.extend(s * 768 + h * 64 + d for d in range(D))
        cols_qk = np.array(cols_qk)
        cols_v = np.array([2 * 768 + hs[i // 64] * 64 + (i % 64) for i in range(384)])
        rows_p = np.array(
            [hs[2 * ko + half] * 64 + d
             for ko in range(3) for half in range(2) for d in range(D)]
        )
        in_maps.append({
            "xT": np.ascontiguousarray(x[b].T),
            "w_qk": np.ascontiguousarray(
                W_qkv[:, cols_qk].reshape(6, P, 768).transpose(1, 0, 2)),
            "w_v": np.ascontiguousarray(
                W_qkv[:, cols_v].reshape(6, P, 384).transpose(1, 0, 2)),
            "w_p": np.ascontiguousarray(
                W_proj[rows_p].reshape(3, P, 768).transpose(1, 0, 2)
            ).astype(ml_dtypes.bfloat16),
            "b_qk": np.ascontiguousarray(b_qkv[cols_qk].reshape(1, 768)),
            "b_qk_t": np.ascontiguousarray(
                b_qkv[cols_qk].reshape(6, P).T),
            "b_v": np.ascontiguousarray(b_qkv[cols_v].reshape(1, 384)),
            "ones": np.ones((1, TC), np.float32),
            "cos_tab": cos_tab,
            "m2s_tab": m2s_tab,
        })
    return in_maps


def kernel(x, rope_cos, rope_sin, W_qkv, b_qkv, W_proj, b_proj, num_special):
    global LAST_RESULTS
    from concourse.bass_utils import run_bass_kernel_spmd

    x = np.asarray(x, np.float32)
    if "nc" not in _NC_CACHE:
        _NC_CACHE["nc"] = _build_nc()
    nc = _NC_CACHE["nc"]

    in_maps = _host_inputs(
        x, np.asarray(rope_cos, np.float32), np.asarray(rope_sin, np.float32),
        np.asarray(W_qkv, np.float32), np.asarray(b_qkv, np.float32),
        np.asarray(W_proj, np.float32), np.asarray(b_proj, np.float32), num_special,
    )
    trace = bool(int(os.environ.get("KERNEL_TRACE", "0")))
    res = run_bass_kernel_spmd(nc, in_maps, core_ids=list(range(8)), trace=trace)
    LAST_RESULTS = res

    bp = np.asarray(b_proj, np.float32)
    out = np.empty((B, N, C), np.float32)
    for b in range(B):
        out[b] = res.results[2 * b]["y"] + res.results[2 * b + 1]["y"] + bp
    return out
